# revision 20
# baseline (speedup 1.0000x reference)
"""Trainium2 Bass kernel for DLUPack (CARAFE-style dynamic upsampling).

Module: 1x1 compress conv -> 3x3 offset/kernel convs -> softmax over 25
kernel channels -> bilinear grid-sample of the mask at offset positions
(2x upsample) -> CARAFE 5x5 reassembly of x with the upsampled mask.

Shapes (hardcoded): x (2,256,64,64) f32 -> out (2,256,128,128) f32.

Sharding: 8 cores = (n in 0..2) x (h-quarter in 0..4). Each core computes
out rows hout in [32*qh, 32*qh+32) for one n. Inputs are sliced/padded
host-side per core; no cross-device communication.

Banded-matrix CARAFE formulation: for output block h (16 per core), the
5x5-tap reassembly is 3 accumulating matmuls per channel half:
  out[c, (p,q,w)] = sum_P  XP_P[(ki,w'), c]^T  @  B_P[(ki,w'), (p,q,w)]
where XP packs two x rows on the partition axis and B_P holds the
grid-sampled mask values on 5 diagonals (w' = w + kj - 2). B tiles are
built by gpsimd local_scatter (per-partition indices) from mask data
computed entirely in w-on-partitions layout; all +-1/+-2 column shifts
(bilinear dx neighbors and band diagonals) are applied by constant
shift-matrix matmuls on TensorE. One 40-row transpose per mask row
covers mask + offsets. The mask->B->matmul back half is pipelined in 4
h-groups with per-group tiles so all five engines and DMA overlap.
Conv biases ride the matmuls as rank-1 (ones-vector) contraction terms.
"""

import os

import numpy as np
import ml_dtypes

import concourse.bass as bass
import concourse.tile as tile
from concourse import bacc, mybir
from concourse.bass_utils import run_bass_kernel_spmd

F32 = mybir.dt.float32
FP16 = mybir.dt.float16
I16 = mybir.dt.int16
ALU = mybir.AluOpType
ACTF = mybir.ActivationFunctionType

N, C, H, W = 2, 256, 64, 64
S, K, CC = 2, 5, 64
HOUT, WOUT = H * S, W * S
QH = 4                 # h-quarters
HB = H // QH           # 16 low-res rows per core
RX = HB + 4            # x rows incl +-2 halo
RM = HB + 2            # mask rows incl +-1 halo
NTAP = 9               # 3x3 conv taps
NP = 3                 # ki pair-tiles: (0,1), (2,3), (4,)
NG = 4                 # h-groups of 4 for the pipelined back half
GH = HB // NG

_cache = {}


def _build():
    nc = bacc.Bacc("TRN2", target_bir_lowering=False, debug=False,
                   num_devices=8)

    def din(name, shape, dt=F32):
        return nc.dram_tensor(name, shape, dt, kind="ExternalInput").ap()

    x_sl = din("x_sl", [C, RX, W])
    xpair = din("xpair", [RX, 128, 256], FP16)
    w1l = din("w1l", [C, CC])
    b1r = din("b1r", [1, CC])
    w2l = din("w2l", [CC, NTAP * 40])
    b2r = din("b2r", [1, 40])
    ident = din("ident", [128, 128])
    c64 = din("c64", [64, 322])      # ylo 64 | yhi 64 | xlo 1 | xhi 1 | dyt 192
    shf = din("shf", [128, 5 * 128], FP16)
    idxc = din("idxc", [128, 80], I16)
    rmask = din("rmask", [CC, RX])
    out_sl = nc.dram_tensor("out_sl", [C, 2 * HB, WOUT], F32,
                            kind="ExternalOutput").ap()
    dbg = {}
    if _cache.get("debug"):
        for nm, sh, dt in [("d_mTE0", [64, RM * 25], F32),
                           ("d_sRT0", [64, RM], F32),
                           ("d_tmpOff", [64, RM * 8], F32),
                           ("d_TRIY", [128, 192], FP16),
                           ("d_TRIX", [128, 192], FP16),
                           ("d_MSN1", [128, RM * 15], FP16),
                           ("d_MSN0", [128, RM * 15], FP16)]:
            dbg[nm] = nc.dram_tensor(nm, sh, dt, kind="ExternalOutput").ap()

    with tile.TileContext(nc) as tc:
        with tc.tile_pool(name="per", bufs=1) as per, \
             tc.tile_pool(name="psA", bufs=1, space="PSUM") as psA, \
             tc.tile_pool(name="psT", bufs=2, space="PSUM") as psT, \
             tc.tile_pool(name="psS", bufs=2, space="PSUM") as psS, \
             tc.tile_pool(name="psB", bufs=3, space="PSUM") as psB:

            # ---------------- persistent tiles ----------------
            XS0 = per.tile([128, RX * W], F32, tag="XS0")
            XS1 = per.tile([128, RX * W], F32, tag="XS1")
            w1a = per.tile([128, CC], F32, tag="w1a")
            w1b = per.tile([128, CC], F32, tag="w1b")
            b1t = per.tile([1, CC], F32, tag="b1t")
            w2t = per.tile([CC, NTAP * 40], F32, tag="w2t")
            b2t = per.tile([1, 40], F32, tag="b2t")
            ones = per.tile([1, 6 * W], F32, tag="ones")
            idt = per.tile([128, 128], F32, tag="idt")
            c64t = per.tile([64, 322], F32, tag="c64t")
            shfT = per.tile([128, 5 * 128], FP16, tag="shfT")
            idxT = per.tile([128, 80], I16, tag="idxT")
            rmt = per.tile([CC, RX], F32, tag="rmt")
            XPB = per.tile([128, RX * 256], FP16, tag="XPB")

            compp = per.tile([CC, RX * (W + 2)], F32, tag="compp")
            EO = per.tile([40, RM * W + 2], F32, tag="EO")
            mTE1 = per.tile([64, RM * 25], F32, tag="mTE1")
            sRT1 = per.tile([64, RM], F32, tag="sRT1")
            tmpOff = per.tile([64, RM * 8], F32, tag="tmpOff")
            MSN = [per.tile([128, RM * 15], FP16, name=f"MSN{d}")
                   for d in range(3)]
            OYC = per.tile([64, 64], F32, tag="OYC")
            OXC = per.tile([64, 64], F32, tag="OXC")
            TRIYf = per.tile([64, 192], F32, tag="TRIYf")
            TRIXf = per.tile([64, 192], F32, tag="TRIXf")
            TRIY = per.tile([128, 192], FP16, tag="TRIY")
            TRIX = per.tile([128, 192], FP16, tag="TRIX")
            tmpW = per.tile([64, 192], F32, tag="tmpW")
            INNER = per.tile([128, NP * GH * 20], FP16, tag="INNER")
            tmpI = per.tile([128, NP * GH * 20], FP16, tag="tmpI")
            tmpJ = per.tile([128, NP * GH * 20], FP16, tag="tmpJ")
            MWg = [per.tile([128, NP * GH * 20], FP16, name=f"MW{g}")
                   for g in range(NG)]
            DMg = [per.tile([128, NP * GH * 20], FP16, name=f"DM{g}")
                   for g in range(NG)]
            Bg = [[per.tile([128, GH * 256], FP16, name=f"B{g}_{P}")
                   for P in range(NP)] for g in range(NG)]
            OCg = [[per.tile([128, GH * 256], F32, name=f"OC{g}_{ch}")
                    for ch in range(2)] for g in range(NG)]

            # views of the packed const tile
            yloT = c64t[:, 0:64]
            yhiT = c64t[:, 64:128]
            xloT = c64t[:, 128:129]
            xhiT = c64t[:, 129:130]
            dytT = c64t[:, 130:322]

            # ---------------- input DMAs ----------------
            xv3 = x_sl.rearrange("c r w -> c (r w)")
            nc.sync.dma_start(XS0[:], xv3[0:128, :])
            nc.sync.dma_start(XS1[:], xv3[128:256, :])
            nc.sync.dma_start(w1a[:], w1l[0:128, :])
            nc.sync.dma_start(w1b[:], w1l[128:256, :])
            nc.sync.dma_start(b1t[:], b1r[:])
            nc.sync.dma_start(w2t[:], w2l[:])
            nc.sync.dma_start(b2t[:], b2r[:])
            nc.sync.dma_start(idt[:], ident[:])
            nc.sync.dma_start(rmt[:], rmask[:])
            nc.sync.dma_start(c64t[:], c64[:])
            nc.scalar.dma_start(XPB[:].rearrange("p (r c) -> p r c", r=RX),
                                xpair.rearrange("r p c -> p r c"))
            nc.scalar.dma_start(shfT[:], shf[:])
            nc.scalar.dma_start(idxT[:], idxc[:])
            nc.vector.memset(ones[:], 1.0)

            # ---------------- conv1 (1x1) ----------------
            nc.vector.memset(compp[:], 0.0)
            cpv = compp[:].rearrange("p (r w) -> p r w", r=RX)
            xs0v = XS0[:].rearrange("p (r w) -> p r w", r=RX)
            xs1v = XS1[:].rearrange("p (r w) -> p r w", r=RX)
            for i in range(4):
                r0 = i * 5
                p1 = psA.tile([CC, 5 * W], F32, tag="cv", name="p1")
                nc.tensor.matmul(p1[:], w1a[:],
                                 xs0v[:, r0:r0 + 5, :], start=True, stop=False)
                nc.tensor.matmul(p1[:], w1b[:],
                                 xs1v[:, r0:r0 + 5, :], start=False, stop=False)
                nc.tensor.matmul(p1[:], b1t[:], ones[:, 0:5 * W],
                                 start=False, stop=True)
                nc.scalar.activation(
                    cpv[:, r0:r0 + 5, 1:65],
                    p1[:].rearrange("p (r w) -> p r w", r=5), ACTF.Copy)
            # zero comp rows outside global [0, H) (per-core 0/1 row mask)
            nc.vector.tensor_tensor(
                cpv[:, :, 1:65],
                cpv[:, :, 1:65],
                rmt[:].unsqueeze(2).broadcast_to([CC, RX, W]),
                op=ALU.mult)

            # ---------------- conv2 (3x3) + exp + off ----------------
            w2v = w2t[:].rearrange("p (t o) -> p t o", t=NTAP)
            eov = EO[:]  # [40, RM*W+2]; data cols at offset 1
            eo25 = EO[0:25, 1:1 + RM * W].rearrange("p (r w) -> p r w", r=RM)
            eo8 = EO[32:40, 1:1 + RM * W].rearrange("p (r w) -> p r w", r=RM)
            for i in range(3):
                r0 = i * 6
                p2 = psA.tile([40, 6 * W], F32, tag="cv", name="p2")
                for t in range(NTAP):
                    dy, dx = t // 3, t % 3
                    nc.tensor.matmul(
                        p2[:].rearrange("p (r w) -> p r w", r=6),
                        w2v[:, t, :],
                        cpv[:, r0 + dy:r0 + dy + 6, dx:dx + W],
                        start=(t == 0), stop=False)
                nc.tensor.matmul(p2[:], b2t[:], ones[:],
                                 start=False, stop=True)
                nc.scalar.activation(
                    eo25[:, r0:r0 + 6, :],
                    p2[0:25, :].rearrange("p (r w) -> p r w", r=6),
                    ACTF.Exp)
                nc.scalar.activation(
                    eo8[:, r0:r0 + 6, :],
                    p2[32:40, :].rearrange("p (r w) -> p r w", r=6),
                    ACTF.Copy)

            # ------------- one 40-row transpose per mask row -------------
            for r in range(RM):
                pt = psT.tile([64, 40], F32, tag="ptc", name="pt")
                nc.tensor.transpose(pt[:], eov[0:40, 1 + r * W:1 + r * W + 64],
                                    idt[0:40, 0:40])
                nc.scalar.activation(mTE1[:, r * 25:(r + 1) * 25],
                                     pt[:, 0:25], ACTF.Copy)
                nc.scalar.activation(tmpOff[:, r * 8:(r + 1) * 8],
                                     pt[:, 32:40], ACTF.Copy)

            # ---------------- softmax normalizer 1/max(sum,1) ----------------
            nc.vector.tensor_reduce(
                sRT1[:].unsqueeze(2),
                mTE1[:].rearrange("p (r c) -> p r c", r=RM),
                axis=mybir.AxisListType.X, op=ALU.add)
            nc.vector.tensor_scalar_max(sRT1[:], sRT1[:], 1.0)
            nc.vector.reciprocal(sRT1[:], sRT1[:])

            # ------- MSN (center): j-packed normalized masks, (r, P, kj) ----
            mtv = mTE1[:].rearrange("p (r k) -> p r k", r=RM)
            srb = sRT1[:].unsqueeze(2).broadcast_to([64, RM, 5])
            msv1 = MSN[1][:].rearrange("p (r P k) -> p r P k", r=RM, P=NP)
            for P in range(NP):
                for j in range(2):
                    k0 = (2 * P + j) * 5 if P < 2 else 20
                    nc.vector.tensor_tensor(
                        msv1[64 * j:64 * j + 64, :, P, :],
                        mtv[:, :, k0:k0 + 5], srb, op=ALU.mult)
            # +-1 column shifts of the center mask via shift matmuls
            for (dxi, kj) in ((0, 3), (2, 1)):
                pm = psS.tile([128, RM * 15], F32, tag="pskj",
                              name=f"pmsn{dxi}")
                nc.tensor.matmul(pm[:], shfT[:, kj * 128:(kj + 1) * 128],
                                 MSN[1][:], start=True, stop=True)
                nc.scalar.activation(MSN[dxi][:], pm[:], ACTF.Copy)

            # ---------------- WGT: bilinear tri-weights ----------------
            # tmpOff[w, (r, q, xy, p)]; rows r=1..17 are h=0..16
            tov = tmpOff[:].rearrange("p (r q xy pp) -> p r q xy pp",
                                      r=RM, q=2, xy=2)
            oyv = tov[:, 1:1 + HB, :, 1, :].transpose([0, 1, 3, 2])
            oxv = tov[:, 1:1 + HB, :, 0, :].transpose([0, 1, 3, 2])
            oycv = OYC[:].rearrange("p (h pp q) -> p h pp q", h=HB, pp=2)
            oxcv = OXC[:].rearrange("p (h pp q) -> p h pp q", h=HB, pp=2)
            nc.vector.tensor_tensor(
                oycv, oyv,
                yloT.rearrange("p (h pp q) -> p h pp q", h=HB, pp=2),
                op=ALU.max)
            nc.vector.tensor_tensor(
                oycv, oycv,
                yhiT.rearrange("p (h pp q) -> p h pp q", h=HB, pp=2),
                op=ALU.min)
            nc.vector.tensor_tensor(
                oxcv, oxv,
                xloT.unsqueeze(2).unsqueeze(3).broadcast_to([64, HB, 2, 2]),
                op=ALU.max)
            nc.vector.tensor_tensor(
                oxcv, oxcv,
                xhiT.unsqueeze(2).unsqueeze(3).broadcast_to([64, HB, 2, 2]),
                op=ALU.min)
            for (trif, tri16, oc) in ((TRIYf, TRIY, OYC), (TRIXf, TRIX, OXC)):
                ocb = oc[:].unsqueeze(1).broadcast_to([64, 3, 64])
                nc.vector.tensor_tensor(
                    tmpW[:].rearrange("p (d f) -> p d f", d=3), ocb,
                    dytT.rearrange("p (d f) -> p d f", d=3),
                    op=ALU.subtract)
                nc.vector.tensor_scalar(trif[:], tmpW[:], -1.0, None,
                                        op0=ALU.mult)
                nc.vector.tensor_tensor(trif[:], trif[:], tmpW[:], op=ALU.max)
                nc.vector.tensor_scalar(trif[:], trif[:], -1.0, 1.0,
                                        op0=ALU.mult, op1=ALU.add)
                nc.vector.tensor_scalar(trif[:], trif[:], 0.0, None,
                                        op0=ALU.max)
                nc.vector.tensor_copy(tri16[0:64, :], trif[:])
                nc.vector.tensor_copy(tri16[64:128, :], trif[:])

            # ---------------- pipelined back half (per h-group) ----------
            trxv = TRIX[:].rearrange("p (d h e) -> p d h e", d=3, h=HB)
            tryv = TRIY[:].rearrange("p (d h e) -> p d h e", d=3, h=HB)
            inv = INNER[:].rearrange("p (h pk e) -> p h pk e",
                                     h=GH, pk=NP * 5)
            ov = out_sl.rearrange("c r w -> c (r w)")
            for g in range(NG):
                hg0 = g * GH
                # --- MW: weighted upsampled mask, all 3 P-tiles per op ---
                # layout (h, (P,k), e=(p,q)) so every operand is <=3 free dims
                mwv = MWg[g][:].rearrange("p (h pk e) -> p h pk e",
                                          h=GH, pk=NP * 5)
                for dyi in range(3):
                    for dxi in range(3):
                        msl = MSN[dxi][:].rearrange(
                            "p (r pk) -> p r pk", r=RM)[
                            :, hg0 + dyi:hg0 + dyi + GH] \
                            .unsqueeze(3).broadcast_to([128, GH, 15, 4])
                        txl = trxv[:, dxi, hg0:hg0 + GH] \
                            .unsqueeze(2).broadcast_to([128, GH, 15, 4])
                        dst = inv if dxi == 0 else tmpI[:].rearrange(
                            "p (h pk e) -> p h pk e", h=GH, pk=15)
                        nc.vector.tensor_tensor(dst, txl, msl, op=ALU.mult)
                        if dxi > 0:
                            nc.vector.tensor_tensor(INNER[:], INNER[:],
                                                    tmpI[:], op=ALU.add)
                    tyl = tryv[:, dyi, hg0:hg0 + GH] \
                        .unsqueeze(2).broadcast_to([128, GH, 15, 4])
                    dst = mwv if dyi == 0 else tmpJ[:].rearrange(
                        "p (h pk e) -> p h pk e", h=GH, pk=15)
                    nc.vector.tensor_tensor(dst, tyl, inv, op=ALU.mult)
                    if dyi > 0:
                        nc.vector.tensor_tensor(MWg[g][:], MWg[g][:],
                                                tmpJ[:], op=ALU.add)

                # --- DM: kj-shift via constant shift-matrix matmuls ---
                mwk = MWg[g][:].rearrange("p (h P k e) -> p h P k e",
                                          h=GH, P=NP, k=5)
                dmk = DMg[g][:].rearrange("p (P h k e) -> p P h k e",
                                          P=NP, h=GH, k=5)
                for kj in range(5):
                    ps = psS.tile([128, NP * GH * 4], F32, tag="pskj",
                                  name=f"ps{g}_{kj}")
                    psv = ps[:].rearrange("p (h P e) -> p h P e",
                                          h=GH, P=NP)
                    nc.tensor.matmul(psv, shfT[:, kj * 128:(kj + 1) * 128],
                                     mwk[:, :, :, kj, :],
                                     start=True, stop=True)
                    nc.scalar.activation(dmk[:, :, :, kj, :],
                                         psv.transpose([0, 2, 1, 3]),
                                         ACTF.Copy)

                # --- B scatter (gpsimd) ---
                for P in range(NP):
                    ch = 128 if P < 2 else 64
                    nc.gpsimd.local_scatter(
                        Bg[g][P][0:ch, :],
                        DMg[g][0:ch, P * GH * 20:(P + 1) * GH * 20],
                        idxT[0:ch, :], ch, GH * 256, GH * 20)

                # --- CARAFE banded matmuls + evac ---
                for hh in range(GH):
                    h = hg0 + hh
                    for ch in range(2):
                        pb = psB.tile([128, 256], F32, tag="pb",
                                      name=f"pb{h}_{ch}")
                        cs = ch * 128
                        nc.tensor.matmul(
                            pb[:], XPB[:, h * 256 + cs:h * 256 + cs + 128],
                            Bg[g][0][:, hh * 256:(hh + 1) * 256],
                            start=True, stop=False)
                        nc.tensor.matmul(
                            pb[:],
                            XPB[:, (h + 2) * 256 + cs:(h + 2) * 256 + cs + 128],
                            Bg[g][1][:, hh * 256:(hh + 1) * 256],
                            start=False, stop=False)
                        nc.tensor.matmul(
                            pb[:],
                            XPB[0:64, (h + 4) * 256 + cs:(h + 4) * 256 + cs + 128],
                            Bg[g][2][0:64, hh * 256:(hh + 1) * 256],
                            start=False, stop=True)
                        dst = OCg[g][ch][:].rearrange(
                            "c (h pp w q) -> c h pp w q",
                            h=GH, pp=2, w=W)[:, hh]
                        nc.scalar.activation(
                            dst,
                            pb[:].rearrange("c (pp q w) -> c pp q w",
                                            pp=2, q=2)
                            .transpose([0, 1, 3, 2]),
                            ACTF.Copy)

                # --- store this group's output rows ---
                csl = slice(g * GH * 256, (g + 1) * GH * 256)
                nc.sync.dma_start(ov[0:128, csl], OCg[g][0][:])
                nc.sync.dma_start(ov[128:256, csl], OCg[g][1][:])

            if _cache.get("debug"):
                nc.sync.dma_start(dbg["d_mTE0"], mTE1[:])
                nc.sync.dma_start(dbg["d_sRT0"], sRT1[:])
                nc.sync.dma_start(dbg["d_tmpOff"], tmpOff[:])
                nc.sync.dma_start(dbg["d_TRIY"], TRIY[:])
                nc.sync.dma_start(dbg["d_TRIX"], TRIX[:])
                nc.sync.dma_start(dbg["d_MSN1"], MSN[1][:])
                nc.sync.dma_start(dbg["d_MSN0"], MSN[0][:])

    nc.compile()
    return nc


def _consts(n, qh):
    h0 = qh * HB
    hg = h0 + np.arange(HB, dtype=np.float32)          # global h per local h
    ylo = np.broadcast_to(np.repeat(-hg, 4)[None, :], (64, 64))
    yhi = np.broadcast_to(np.repeat(63.0 - hg, 4)[None, :], (64, 64))
    wv = np.arange(W, dtype=np.float32)
    xlo = (-wv)[:, None]
    xhi = (63.0 - wv)[:, None]
    dyv = np.array([-1.0, 0.0, 1.0], np.float32)
    dyt = np.broadcast_to(np.repeat(dyv, 64)[None, :], (64, 192))
    c64 = np.concatenate([ylo, yhi, xlo, xhi, dyt], axis=1).astype(np.float32)
    # shift matrices: SHF[kj][p, m] = 1 iff same 64-block, m%64 = p%64 + kj-2
    shf = np.zeros((128, 5, 128), np.float16)
    for kj in range(5):
        for p in range(128):
            m = p + kj - 2
            if p // 64 == m // 64 and 0 <= m < 128:
                shf[p, kj, m] = 1.0
    shf = shf.reshape(128, 5 * 128)
    # scatter indices: partition (j,w'), slot (hh, kj, p, q) ->
    # col hh*256 + p*128 + q*64 + (w'+2-kj), or -1 if w out of range
    idx = np.full((128, 80), -1, np.int16)
    for pp in range(128):
        wp = pp % 64
        for hh in range(4):
            for kj in range(5):
                for p in range(2):
                    for q in range(2):
                        w = wp + 2 - kj
                        if 0 <= w < W:
                            idx[pp, hh * 20 + kj * 4 + p * 2 + q] = (
                                hh * 256 + p * 128 + q * 64 + w)
    rm = np.zeros((CC, RX), np.float32)
    for r in range(RX):
        g = h0 - 2 + r
        rm[:, r] = 1.0 if 0 <= g < H else 0.0
    return dict(c64=np.ascontiguousarray(c64), shf=shf, idxc=idx, rmask=rm)


def kernel(x, w_comp, b_comp, w_off, b_off, w_ker, b_ker):
    x = np.asarray(x, np.float32)
    w_comp = np.asarray(w_comp, np.float32)
    b_comp = np.asarray(b_comp, np.float32)
    w_off = np.asarray(w_off, np.float32)
    b_off = np.asarray(b_off, np.float32)
    w_ker = np.asarray(w_ker, np.float32)
    b_ker = np.asarray(b_ker, np.float32)

    if "nc" not in _cache:
        _cache["nc"] = _build()
    nc = _cache["nc"]

    w1l = w_comp.reshape(CC, C).T.copy()
    perm = [xy * 4 + p * 2 + q for q in range(2) for xy in range(2)
            for p in range(2)]
    w2 = np.zeros((40, CC, 3, 3), np.float32)
    b2 = np.zeros((40,), np.float32)
    w2[0:25] = w_ker
    b2[0:25] = b_ker
    w2[32:40] = w_off[perm]
    b2[32:40] = b_off[perm]
    w2l = np.ascontiguousarray(
        w2.transpose(1, 2, 3, 0).reshape(CC, NTAP * 40))   # [cc, (tap, oc)]
    ident = np.eye(128, dtype=np.float32)

    in_maps = []
    for core in range(8):
        n, qh = core // QH, core % QH
        h0 = qh * HB
        xs = np.zeros((C, RX, W), np.float32)
        lo, hi = h0 - 2, h0 + HB + 2
        slo, shi = max(lo, 0), min(hi, H)
        xs[:, slo - lo:shi - lo] = x[n, :, slo:shi]
        xt = np.ascontiguousarray(xs.transpose(2, 1, 0)).astype(np.float16)
        xp = np.zeros((RX, 128, 256), np.float16)
        xp[:, 0:64, :] = xt.transpose(1, 0, 2)
        xp[:RX - 1, 64:128, :] = xt.transpose(1, 0, 2)[1:]
        im = dict(x_sl=xs, xpair=xp, w1l=w1l,
                  b1r=b_comp[None, :].copy(), b2r=b2[None, :].copy(),
                  w2l=w2l, ident=ident,
                  **_consts(n, qh))
        in_maps.append(im)

    res = run_bass_kernel_spmd(nc, in_maps, core_ids=list(range(8)),
                               trace=bool(os.environ.get("DLU_TRACE")))
    _cache["last_res"] = res
    out = np.zeros((N, C, HOUT, WOUT), np.float32)
    for core in range(8):
        n, qh = core // QH, core % QH
        out[n, :, 2 * qh * HB:2 * (qh + 1) * HB] = res.results[core]["out_sl"]
    return out


# revision 25
# speedup vs baseline: 1.3877x; 1.3877x over previous
"""Trainium2 Bass kernel for DLUPack (CARAFE-style dynamic upsampling).

Module: 1x1 compress conv -> 3x3 offset/kernel convs -> softmax over 25
kernel channels -> bilinear grid-sample of the mask at offset positions
(2x upsample) -> CARAFE 5x5 reassembly of x with the upsampled mask.

Shapes (hardcoded): x (2,256,64,64) f32 -> out (2,256,128,128) f32.

Sharding: 8 cores = (n in 0..2) x (h-quarter in 0..4). Each core computes
out rows hout in [32*qh, 32*qh+32) for one n. Inputs are sliced/padded
host-side per core; no cross-device communication.

Banded-matrix CARAFE formulation: for output block h (16 per core), the
5x5-tap reassembly is 3 accumulating matmuls per channel half:
  out[c, (p,q,w)] = sum_P  XP_P[(ki,w'), c]^T  @  B_P[(ki,w'), (p,q,w)]
where XP packs two x rows on the partition axis and B_P holds the
grid-sampled mask values on 5 diagonals (w' = w + kj - 2). B tiles are
built by gpsimd local_scatter (per-partition indices) from mask data
computed entirely in w-on-partitions layout; all +-1/+-2 column shifts
(bilinear dx neighbors and band diagonals) are applied by constant
shift-matrix matmuls on TensorE. One 40-row transpose per mask row
covers mask + offsets. The mask->B->matmul back half is pipelined in 4
h-groups with per-group tiles so all five engines and DMA overlap.
Conv biases ride the matmuls as rank-1 (ones-vector) contraction terms.
"""

import os

import numpy as np
import ml_dtypes

import concourse.bass as bass
import concourse.tile as tile
from concourse import bacc, mybir
from concourse.bass_utils import run_bass_kernel_spmd

F32 = mybir.dt.float32
FP16 = mybir.dt.float16
I16 = mybir.dt.int16
ALU = mybir.AluOpType
ACTF = mybir.ActivationFunctionType

N, C, H, W = 2, 256, 64, 64
S, K, CC = 2, 5, 64
HOUT, WOUT = H * S, W * S
QH = 4                 # h-quarters
HB = H // QH           # 16 low-res rows per core
RX = HB + 4            # x rows incl +-2 halo
RM = HB + 2            # mask rows incl +-1 halo
NTAP = 9               # 3x3 conv taps
NP = 3                 # ki pair-tiles: (0,1), (2,3), (4,)
NG = 4                 # h-groups of 4 for the pipelined back half
GH = HB // NG

_cache = {}


def _build():
    nc = bacc.Bacc("TRN2", target_bir_lowering=False, debug=False,
                   num_devices=8)

    def din(name, shape, dt=F32):
        return nc.dram_tensor(name, shape, dt, kind="ExternalInput").ap()

    x_sl = din("x_sl", [C, RX, W], FP16)
    xpair = din("xpair", [RX, 128, 256], FP16)
    w1l = din("w1l", [C, CC], FP16)
    b1r = din("b1r", [1, CC], FP16)
    w2l = din("w2l", [CC, NTAP * 40], FP16)
    b2r = din("b2r", [1, 40], FP16)
    ident = din("ident", [128, 128])
    c64 = din("c64", [64, 322])      # ylo 64 | yhi 64 | xlo 1 | xhi 1 | dyt 192
    shf = din("shf", [128, 5 * 128], FP16)
    idxc = din("idxc", [128, 80], I16)
    rmask = din("rmask", [CC, RX], FP16)
    out_sl = nc.dram_tensor("out_sl", [C, 2 * HB, WOUT], FP16,
                            kind="ExternalOutput").ap()
    dbg = {}
    if _cache.get("debug"):
        for nm, sh, dt in [("d_mTE0", [64, RM * 25], FP16),
                           ("d_sRT0", [64, RM], F32),
                           ("d_tmpOff", [64, RM * 8], F32),
                           ("d_TRIY", [128, 192], FP16),
                           ("d_TRIX", [128, 192], FP16),
                           ("d_MSN1", [128, RM * 15], FP16),
                           ("d_MSN0", [128, RM * 15], FP16)]:
            dbg[nm] = nc.dram_tensor(nm, sh, dt, kind="ExternalOutput").ap()

    with tile.TileContext(nc) as tc:
        with tc.tile_pool(name="per", bufs=1) as per, \
             tc.tile_pool(name="psA", bufs=1, space="PSUM") as psA, \
             tc.tile_pool(name="psT", bufs=2, space="PSUM") as psT, \
             tc.tile_pool(name="psS", bufs=2, space="PSUM") as psS, \
             tc.tile_pool(name="psB", bufs=3, space="PSUM") as psB:

            # ---------------- persistent tiles ----------------
            XS0 = per.tile([128, RX * W], FP16, tag="XS0")
            XS1 = per.tile([128, RX * W], FP16, tag="XS1")
            w1a = per.tile([128, CC], FP16, tag="w1a")
            w1b = per.tile([128, CC], FP16, tag="w1b")
            b1t = per.tile([1, CC], FP16, tag="b1t")
            w2t = per.tile([CC, NTAP * 40], FP16, tag="w2t")
            b2t = per.tile([1, 40], FP16, tag="b2t")
            ones = per.tile([1, 6 * W], FP16, tag="ones")
            idt16 = per.tile([128, 128], FP16, tag="idt16")
            idt = per.tile([128, 128], F32, tag="idt")
            c64t = per.tile([64, 322], F32, tag="c64t")
            shfT = per.tile([128, 5 * 128], FP16, tag="shfT")
            idxT = per.tile([128, 80], I16, tag="idxT")
            rmt = per.tile([CC, RX], FP16, tag="rmt")
            XPB = per.tile([128, RX * 256], FP16, tag="XPB")

            compp = per.tile([CC, RX * (W + 2)], FP16, tag="compp")
            EO = per.tile([40, RM * W + 2], FP16, tag="EO")
            mTE1 = per.tile([64, RM * 25], FP16, tag="mTE1")
            sRT1 = per.tile([64, RM], F32, tag="sRT1")
            tmpOff = per.tile([64, RM * 8], F32, tag="tmpOff")
            MSN = [per.tile([128, RM * 15], FP16, name=f"MSN{d}")
                   for d in range(3)]
            OYC = per.tile([64, 64], F32, tag="OYC")
            OXC = per.tile([64, 64], F32, tag="OXC")
            TRIYf = per.tile([64, 192], F32, tag="TRIYf")
            TRIXf = per.tile([64, 192], F32, tag="TRIXf")
            TRIY = per.tile([128, 192], FP16, tag="TRIY")
            TRIX = per.tile([128, 192], FP16, tag="TRIX")
            tmpW = per.tile([64, 192], F32, tag="tmpW")
            INNER = per.tile([128, NP * GH * 20], FP16, tag="INNER")
            tmpI = per.tile([128, NP * GH * 20], FP16, tag="tmpI")
            tmpJ = per.tile([128, NP * GH * 20], FP16, tag="tmpJ")
            MWg = [per.tile([128, NP * GH * 20], FP16, name=f"MW{g}")
                   for g in range(NG)]
            DMg = [per.tile([128, NP * GH * 20], FP16, name=f"DM{g}")
                   for g in range(NG)]
            Bg = [[per.tile([128, GH * 256], FP16, name=f"B{g}_{P}")
                   for P in range(NP)] for g in range(NG)]
            OCg = [[per.tile([128, GH * 256], FP16, name=f"OC{g}_{ch}")
                    for ch in range(2)] for g in range(NG)]

            # views of the packed const tile
            yloT = c64t[:, 0:64]
            yhiT = c64t[:, 64:128]
            xloT = c64t[:, 128:129]
            xhiT = c64t[:, 129:130]
            dytT = c64t[:, 130:322]

            # ---------------- input DMAs ----------------
            xv3 = x_sl.rearrange("c r w -> c (r w)")
            nc.sync.dma_start(XS0[:], xv3[0:128, :])
            nc.sync.dma_start(XS1[:], xv3[128:256, :])
            nc.sync.dma_start(w1a[:], w1l[0:128, :])
            nc.sync.dma_start(w1b[:], w1l[128:256, :])
            nc.sync.dma_start(b1t[:], b1r[:])
            nc.scalar.dma_start(w2t[:], w2l[:])
            nc.scalar.dma_start(b2t[:], b2r[:])
            nc.sync.dma_start(idt[:], ident[:])
            nc.sync.dma_start(rmt[:], rmask[:])
            nc.sync.dma_start(c64t[:], c64[:])
            nc.vector.tensor_copy(idt16[:], idt[:])
            nc.scalar.dma_start(XPB[:].rearrange("p (r c) -> p r c", r=RX),
                                xpair.rearrange("r p c -> p r c"))
            nc.scalar.dma_start(shfT[:], shf[:])
            nc.scalar.dma_start(idxT[:], idxc[:])
            nc.vector.memset(ones[:], 1.0)

            # ---------------- conv1 (1x1) ----------------
            nc.vector.memset(compp[:], 0.0)
            cpv = compp[:].rearrange("p (r w) -> p r w", r=RX)
            xs0v = XS0[:].rearrange("p (r w) -> p r w", r=RX)
            xs1v = XS1[:].rearrange("p (r w) -> p r w", r=RX)
            for i in range(4):
                r0 = i * 5
                p1 = psA.tile([CC, 5 * W], F32, tag="cv", name="p1")
                nc.tensor.matmul(p1[:], w1a[:],
                                 xs0v[:, r0:r0 + 5, :], start=True, stop=False)
                nc.tensor.matmul(p1[:], w1b[:],
                                 xs1v[:, r0:r0 + 5, :], start=False, stop=False)
                nc.tensor.matmul(p1[:], b1t[:], ones[:, 0:5 * W],
                                 start=False, stop=True)
                nc.scalar.activation(
                    cpv[:, r0:r0 + 5, 1:65],
                    p1[:].rearrange("p (r w) -> p r w", r=5), ACTF.Copy)
            # zero comp rows outside global [0, H) (per-core 0/1 row mask)
            nc.vector.tensor_tensor(
                cpv[:, :, 1:65],
                cpv[:, :, 1:65],
                rmt[:].unsqueeze(2).broadcast_to([CC, RX, W]),
                op=ALU.mult)

            # ---------------- conv2 (3x3) + exp + off ----------------
            w2v = w2t[:].rearrange("p (t o) -> p t o", t=NTAP)
            eov = EO[:]  # [40, RM*W+2]; data cols at offset 1
            eo25 = EO[0:25, 1:1 + RM * W].rearrange("p (r w) -> p r w", r=RM)
            eo8 = EO[32:40, 1:1 + RM * W].rearrange("p (r w) -> p r w", r=RM)
            for i in range(3):
                r0 = i * 6
                p2 = psA.tile([40, 6 * W], F32, tag="cv", name="p2")
                for t in range(NTAP):
                    dy, dx = t // 3, t % 3
                    nc.tensor.matmul(
                        p2[:].rearrange("p (r w) -> p r w", r=6),
                        w2v[:, t, :],
                        cpv[:, r0 + dy:r0 + dy + 6, dx:dx + W],
                        start=(t == 0), stop=False)
                nc.tensor.matmul(p2[:], b2t[:], ones[:],
                                 start=False, stop=True)
                nc.scalar.activation(
                    eo25[:, r0:r0 + 6, :],
                    p2[0:25, :].rearrange("p (r w) -> p r w", r=6),
                    ACTF.Exp)
                nc.scalar.activation(
                    eo8[:, r0:r0 + 6, :],
                    p2[32:40, :].rearrange("p (r w) -> p r w", r=6),
                    ACTF.Copy)

            # --------- one 40x128 transpose per PAIR of mask rows ---------
            for r2 in range(RM // 2):
                r = 2 * r2
                pt = psT.tile([128, 40], FP16, tag="ptc", name="pt")
                nc.tensor.transpose(pt[:],
                                    eov[0:40, 1 + r * W:1 + (r + 2) * W],
                                    idt16[0:40, 0:40])
                nc.scalar.activation(mTE1[:, r * 25:(r + 1) * 25],
                                     pt[0:64, 0:25], ACTF.Copy)
                nc.scalar.activation(mTE1[:, (r + 1) * 25:(r + 2) * 25],
                                     pt[64:128, 0:25], ACTF.Copy)
                nc.vector.tensor_copy(tmpOff[:, r * 8:(r + 1) * 8],
                                      pt[0:64, 32:40])
                nc.vector.tensor_copy(tmpOff[:, (r + 1) * 8:(r + 2) * 8],
                                      pt[64:128, 32:40])

            # ---------------- softmax normalizer 1/max(sum,1) ----------------
            nc.vector.tensor_reduce(
                sRT1[:].unsqueeze(2),
                mTE1[:].rearrange("p (r c) -> p r c", r=RM),
                axis=mybir.AxisListType.X, op=ALU.add)
            nc.vector.tensor_scalar_max(sRT1[:], sRT1[:], 1.0)
            nc.vector.reciprocal(sRT1[:], sRT1[:])

            # ------- MSN (center): j-packed normalized masks, (r, P, kj) ----
            mtv = mTE1[:].rearrange("p (r k) -> p r k", r=RM)
            srb = sRT1[:].unsqueeze(2).broadcast_to([64, RM, 5])
            msv1 = MSN[1][:].rearrange("p (r P k) -> p r P k", r=RM, P=NP)
            for P in range(NP):
                for j in range(2):
                    k0 = (2 * P + j) * 5 if P < 2 else 20
                    nc.vector.tensor_tensor(
                        msv1[64 * j:64 * j + 64, :, P, :],
                        mtv[:, :, k0:k0 + 5], srb, op=ALU.mult)
            # +-1 column shifts of the center mask via shift matmuls
            for (dxi, kj) in ((0, 3), (2, 1)):
                pm = psS.tile([128, RM * 15], F32, tag="pskj",
                              name=f"pmsn{dxi}")
                nc.tensor.matmul(pm[:], shfT[:, kj * 128:(kj + 1) * 128],
                                 MSN[1][:], start=True, stop=True)
                nc.scalar.activation(MSN[dxi][:], pm[:], ACTF.Copy)

            # ---------------- WGT: bilinear tri-weights ----------------
            # tmpOff[w, (r, q, xy, p)]; rows r=1..17 are h=0..16
            tov = tmpOff[:].rearrange("p (r q xy pp) -> p r q xy pp",
                                      r=RM, q=2, xy=2)
            oyv = tov[:, 1:1 + HB, :, 1, :].transpose([0, 1, 3, 2])
            oxv = tov[:, 1:1 + HB, :, 0, :].transpose([0, 1, 3, 2])
            oycv = OYC[:].rearrange("p (h pp q) -> p h pp q", h=HB, pp=2)
            oxcv = OXC[:].rearrange("p (h pp q) -> p h pp q", h=HB, pp=2)
            nc.vector.tensor_tensor(
                oycv, oyv,
                yloT.rearrange("p (h pp q) -> p h pp q", h=HB, pp=2),
                op=ALU.max)
            nc.vector.tensor_tensor(
                oycv, oycv,
                yhiT.rearrange("p (h pp q) -> p h pp q", h=HB, pp=2),
                op=ALU.min)
            nc.vector.tensor_tensor(
                oxcv, oxv,
                xloT.unsqueeze(2).unsqueeze(3).broadcast_to([64, HB, 2, 2]),
                op=ALU.max)
            nc.vector.tensor_tensor(
                oxcv, oxcv,
                xhiT.unsqueeze(2).unsqueeze(3).broadcast_to([64, HB, 2, 2]),
                op=ALU.min)
            for (trif, tri16, oc) in ((TRIYf, TRIY, OYC), (TRIXf, TRIX, OXC)):
                ocb = oc[:].unsqueeze(1).broadcast_to([64, 3, 64])
                nc.vector.tensor_tensor(
                    tmpW[:].rearrange("p (d f) -> p d f", d=3), ocb,
                    dytT.rearrange("p (d f) -> p d f", d=3),
                    op=ALU.subtract)
                nc.vector.tensor_scalar(trif[:], tmpW[:], -1.0, None,
                                        op0=ALU.mult)
                nc.vector.tensor_tensor(trif[:], trif[:], tmpW[:], op=ALU.max)
                nc.vector.tensor_scalar(trif[:], trif[:], -1.0, 1.0,
                                        op0=ALU.mult, op1=ALU.add)
                nc.vector.tensor_scalar(trif[:], trif[:], 0.0, None,
                                        op0=ALU.max)
                nc.vector.tensor_copy(tri16[0:64, :], trif[:])
                nc.vector.tensor_copy(tri16[64:128, :], trif[:])

            # ---------------- pipelined back half (per h-group) ----------
            trxv = TRIX[:].rearrange("p (d h e) -> p d h e", d=3, h=HB)
            tryv = TRIY[:].rearrange("p (d h e) -> p d h e", d=3, h=HB)
            inv = INNER[:].rearrange("p (h pk e) -> p h pk e",
                                     h=GH, pk=NP * 5)
            ov = out_sl.rearrange("c r w -> c (r w)")
            for g in range(NG):
                hg0 = g * GH
                # --- MW: weighted upsampled mask, all 3 P-tiles per op ---
                # layout (h, (P,k), e=(p,q)) so every operand is <=3 free dims
                mwv = MWg[g][:].rearrange("p (h pk e) -> p h pk e",
                                          h=GH, pk=NP * 5)
                for dyi in range(3):
                    for dxi in range(3):
                        msl = MSN[dxi][:].rearrange(
                            "p (r pk) -> p r pk", r=RM)[
                            :, hg0 + dyi:hg0 + dyi + GH] \
                            .unsqueeze(3).broadcast_to([128, GH, 15, 4])
                        txl = trxv[:, dxi, hg0:hg0 + GH] \
                            .unsqueeze(2).broadcast_to([128, GH, 15, 4])
                        dst = inv if dxi == 0 else tmpI[:].rearrange(
                            "p (h pk e) -> p h pk e", h=GH, pk=15)
                        nc.vector.tensor_tensor(dst, txl, msl, op=ALU.mult)
                        if dxi > 0:
                            nc.vector.tensor_tensor(INNER[:], INNER[:],
                                                    tmpI[:], op=ALU.add)
                    tyl = tryv[:, dyi, hg0:hg0 + GH] \
                        .unsqueeze(2).broadcast_to([128, GH, 15, 4])
                    dst = mwv if dyi == 0 else tmpJ[:].rearrange(
                        "p (h pk e) -> p h pk e", h=GH, pk=15)
                    nc.vector.tensor_tensor(dst, tyl, inv, op=ALU.mult)
                    if dyi > 0:
                        nc.vector.tensor_tensor(MWg[g][:], MWg[g][:],
                                                tmpJ[:], op=ALU.add)

                # --- DM: kj-shift via constant shift-matrix matmuls ---
                mwk = MWg[g][:].rearrange("p (h P k e) -> p h P k e",
                                          h=GH, P=NP, k=5)
                dmk = DMg[g][:].rearrange("p (P h k e) -> p P h k e",
                                          P=NP, h=GH, k=5)
                for kj in range(5):
                    ps = psS.tile([128, NP * GH * 4], F32, tag="pskj",
                                  name=f"ps{g}_{kj}")
                    psv = ps[:].rearrange("p (h P e) -> p h P e",
                                          h=GH, P=NP)
                    nc.tensor.matmul(psv, shfT[:, kj * 128:(kj + 1) * 128],
                                     mwk[:, :, :, kj, :],
                                     start=True, stop=True)
                    nc.scalar.activation(dmk[:, :, :, kj, :],
                                         psv.transpose([0, 2, 1, 3]),
                                         ACTF.Copy)

                # --- B scatter (gpsimd) ---
                for P in range(NP):
                    ch = 128 if P < 2 else 64
                    nc.gpsimd.local_scatter(
                        Bg[g][P][0:ch, :],
                        DMg[g][0:ch, P * GH * 20:(P + 1) * GH * 20],
                        idxT[0:ch, :], ch, GH * 256, GH * 20)

                # --- CARAFE banded matmuls + evac ---
                for hh in range(GH):
                    h = hg0 + hh
                    for ch in range(2):
                        pb = psB.tile([128, 256], F32, tag="pb",
                                      name=f"pb{h}_{ch}")
                        cs = ch * 128
                        nc.tensor.matmul(
                            pb[:], XPB[:, h * 256 + cs:h * 256 + cs + 128],
                            Bg[g][0][:, hh * 256:(hh + 1) * 256],
                            start=True, stop=False)
                        nc.tensor.matmul(
                            pb[:],
                            XPB[:, (h + 2) * 256 + cs:(h + 2) * 256 + cs + 128],
                            Bg[g][1][:, hh * 256:(hh + 1) * 256],
                            start=False, stop=False)
                        nc.tensor.matmul(
                            pb[:],
                            XPB[0:64, (h + 4) * 256 + cs:(h + 4) * 256 + cs + 128],
                            Bg[g][2][0:64, hh * 256:(hh + 1) * 256],
                            start=False, stop=True)
                        dst = OCg[g][ch][:].rearrange(
                            "c (h pp w q) -> c h pp w q",
                            h=GH, pp=2, w=W)[:, hh]
                        nc.scalar.activation(
                            dst,
                            pb[:].rearrange("c (pp q w) -> c pp q w",
                                            pp=2, q=2)
                            .transpose([0, 1, 3, 2]),
                            ACTF.Copy)

                # --- store this group's output rows ---
                csl = slice(g * GH * 256, (g + 1) * GH * 256)
                nc.sync.dma_start(ov[0:128, csl], OCg[g][0][:])
                nc.sync.dma_start(ov[128:256, csl], OCg[g][1][:])

            if _cache.get("debug"):
                nc.sync.dma_start(dbg["d_mTE0"], mTE1[:])
                nc.sync.dma_start(dbg["d_sRT0"], sRT1[:])
                nc.sync.dma_start(dbg["d_tmpOff"], tmpOff[:])
                nc.sync.dma_start(dbg["d_TRIY"], TRIY[:])
                nc.sync.dma_start(dbg["d_TRIX"], TRIX[:])
                nc.sync.dma_start(dbg["d_MSN1"], MSN[1][:])
                nc.sync.dma_start(dbg["d_MSN0"], MSN[0][:])

    nc.compile()
    return nc


def _consts(n, qh):
    h0 = qh * HB
    hg = h0 + np.arange(HB, dtype=np.float32)          # global h per local h
    ylo = np.broadcast_to(np.repeat(-hg, 4)[None, :], (64, 64))
    yhi = np.broadcast_to(np.repeat(63.0 - hg, 4)[None, :], (64, 64))
    wv = np.arange(W, dtype=np.float32)
    xlo = (-wv)[:, None]
    xhi = (63.0 - wv)[:, None]
    dyv = np.array([-1.0, 0.0, 1.0], np.float32)
    dyt = np.broadcast_to(np.repeat(dyv, 64)[None, :], (64, 192))
    c64 = np.concatenate([ylo, yhi, xlo, xhi, dyt], axis=1).astype(np.float32)
    # shift matrices: SHF[kj][p, m] = 1 iff same 64-block, m%64 = p%64 + kj-2
    shf = np.zeros((128, 5, 128), np.float16)
    for kj in range(5):
        for p in range(128):
            m = p + kj - 2
            if p // 64 == m // 64 and 0 <= m < 128:
                shf[p, kj, m] = 1.0
    shf = shf.reshape(128, 5 * 128)
    # scatter indices: partition (j,w'), slot (hh, kj, p, q) ->
    # col hh*256 + p*128 + q*64 + (w'+2-kj), or -1 if w out of range
    idx = np.full((128, 80), -1, np.int16)
    for pp in range(128):
        wp = pp % 64
        for hh in range(4):
            for kj in range(5):
                for p in range(2):
                    for q in range(2):
                        w = wp + 2 - kj
                        if 0 <= w < W:
                            idx[pp, hh * 20 + kj * 4 + p * 2 + q] = (
                                hh * 256 + p * 128 + q * 64 + w)
    rm = np.zeros((CC, RX), np.float16)
    for r in range(RX):
        g = h0 - 2 + r
        rm[:, r] = 1.0 if 0 <= g < H else 0.0
    return dict(c64=np.ascontiguousarray(c64), shf=shf, idxc=idx, rmask=rm)


def _prep_inmaps(x, w_comp, b_comp, w_off, b_off, w_ker, b_ker):
    x = np.asarray(x, np.float32)
    w_comp = np.asarray(w_comp, np.float32)
    b_comp = np.asarray(b_comp, np.float32)
    w_off = np.asarray(w_off, np.float32)
    b_off = np.asarray(b_off, np.float32)
    w_ker = np.asarray(w_ker, np.float32)
    b_ker = np.asarray(b_ker, np.float32)

    w1l = w_comp.reshape(CC, C).T.astype(np.float16)
    perm = [xy * 4 + p * 2 + q for q in range(2) for xy in range(2)
            for p in range(2)]
    w2 = np.zeros((40, CC, 3, 3), np.float32)
    b2 = np.zeros((40,), np.float32)
    w2[0:25] = w_ker
    b2[0:25] = b_ker
    w2[32:40] = w_off[perm]
    b2[32:40] = b_off[perm]
    w2l = np.ascontiguousarray(
        w2.transpose(1, 2, 3, 0).reshape(CC, NTAP * 40)).astype(
        np.float16)   # [cc, (tap, oc)]
    ident = np.eye(128, dtype=np.float32)

    in_maps = []
    for core in range(8):
        n, qh = core // QH, core % QH
        h0 = qh * HB
        xs = np.zeros((C, RX, W), np.float16)
        lo, hi = h0 - 2, h0 + HB + 2
        slo, shi = max(lo, 0), min(hi, H)
        xs[:, slo - lo:shi - lo] = x[n, :, slo:shi].astype(np.float16)
        xt = np.ascontiguousarray(xs.transpose(2, 1, 0))
        xp = np.zeros((RX, 128, 256), np.float16)
        xp[:, 0:64, :] = xt.transpose(1, 0, 2)
        xp[:RX - 1, 64:128, :] = xt.transpose(1, 0, 2)[1:]
        im = dict(x_sl=xs, xpair=xp, w1l=w1l,
                  b1r=b_comp[None, :].astype(np.float16),
                  b2r=b2[None, :].astype(np.float16),
                  w2l=w2l, ident=ident,
                  **_consts(n, qh))
        in_maps.append(im)
    return in_maps


def kernel(x, w_comp, b_comp, w_off, b_off, w_ker, b_ker):
    if "nc" not in _cache:
        _cache["nc"] = _build()
    nc = _cache["nc"]
    in_maps = _prep_inmaps(x, w_comp, b_comp, w_off, b_off, w_ker, b_ker)

    res = run_bass_kernel_spmd(nc, in_maps, core_ids=list(range(8)),
                               trace=bool(os.environ.get("DLU_TRACE")))
    _cache["last_res"] = res
    out = np.zeros((N, C, HOUT, WOUT), np.float32)
    for core in range(8):
        n, qh = core // QH, core % QH
        out[n, :, 2 * qh * HB:2 * (qh + 1) * HB] = \
            res.results[core]["out_sl"].astype(np.float32)
    return out


# revision 28
# speedup vs baseline: 1.8265x; 1.3163x over previous
"""Trainium2 Bass kernel for DLUPack (CARAFE-style dynamic upsampling).

Module: 1x1 compress conv -> 3x3 offset/kernel convs -> softmax over 25
kernel channels -> bilinear grid-sample of the mask at offset positions
(2x upsample) -> CARAFE 5x5 reassembly of x with the upsampled mask.

Shapes (hardcoded): x (2,256,64,64) f32 -> out (2,256,128,128) f32.

Sharding: 8 cores = (n in 0..2) x (h-quarter in 0..4). Each core computes
out rows hout in [32*qh, 32*qh+32) for one n. Inputs are sliced/padded
host-side per core; no cross-device communication.

Banded-matrix CARAFE formulation: for output block h (16 per core), the
5x5-tap reassembly is 3 accumulating matmuls per channel half:
  out[c, (p,q,w)] = sum_P  XP_P[(ki,w'), c]^T  @  B_P[(ki,w'), (p,q,w)]
where XP packs two x rows on the partition axis and B_P holds the
grid-sampled mask values on 5 diagonals (w' = w + kj - 2). B tiles are
built by gpsimd local_scatter (per-partition indices) from mask data
computed entirely in w-on-partitions layout; all +-1/+-2 column shifts
(bilinear dx neighbors and band diagonals) are applied by constant
shift-matrix matmuls on TensorE. One 40-row transpose per mask row
covers mask + offsets. The mask->B->matmul back half is pipelined in 4
h-groups with per-group tiles so all five engines and DMA overlap.
Conv biases ride the matmuls as rank-1 (ones-vector) contraction terms.
"""

import os

import numpy as np
import ml_dtypes

import concourse.bass as bass
import concourse.tile as tile
from concourse import bacc, mybir
from concourse.bass_utils import run_bass_kernel_spmd

F32 = mybir.dt.float32
FP16 = mybir.dt.float16
I16 = mybir.dt.int16
ALU = mybir.AluOpType
ACTF = mybir.ActivationFunctionType

N, C, H, W = 2, 256, 64, 64
S, K, CC = 2, 5, 64
HOUT, WOUT = H * S, W * S
QH = 4                 # h-quarters
HB = H // QH           # 16 low-res rows per core
RX = HB + 4            # x rows incl +-2 halo
RM = HB + 2            # mask rows incl +-1 halo
NTAP = 9               # 3x3 conv taps
NP = 3                 # ki pair-tiles: (0,1), (2,3), (4,)
NG = 4                 # h-groups of 4 for the pipelined back half
GH = HB // NG

_cache = {}


def _build():
    nc = bacc.Bacc("TRN2", target_bir_lowering=False, debug=False,
                   num_devices=8)

    def din(name, shape, dt=F32):
        return nc.dram_tensor(name, shape, dt, kind="ExternalInput").ap()

    x_sl = din("x_sl", [C, RX, W], FP16)
    xpair = din("xpair", [RX, 128, 256], FP16)
    w1l = din("w1l", [C, CC], FP16)
    b1r = din("b1r", [1, CC], FP16)
    w2l = din("w2l", [CC, NTAP * 40], FP16)
    b2r = din("b2r", [1, 40], FP16)
    ident = din("ident", [128, 128])
    c64 = din("c64", [64, 322])      # ylo 64 | yhi 64 | xlo 1 | xhi 1 | dyt 192
    shf = din("shf", [128, 5 * 128], FP16)
    idxc = din("idxc", [128, 80], I16)
    rmask = din("rmask", [CC, RX], FP16)
    out_sl = nc.dram_tensor("out_sl", [C, 2 * HB, WOUT], FP16,
                            kind="ExternalOutput").ap()
    dbg = {}
    if _cache.get("debug"):
        for nm, sh, dt in [("d_mTE0", [64, RM * 25], FP16),
                           ("d_sRT0", [64, RM], F32),
                           ("d_tmpOff", [64, RM * 8], F32),
                           ("d_TRIY", [128, 192], FP16),
                           ("d_TRIX", [128, 192], FP16),
                           ("d_MSN1", [128, RM * 15], FP16),
                           ("d_MSN0", [128, RM * 15], FP16)]:
            dbg[nm] = nc.dram_tensor(nm, sh, dt, kind="ExternalOutput").ap()

    with tile.TileContext(nc) as tc:
        with tc.tile_pool(name="per", bufs=1) as per, \
             tc.tile_pool(name="psA", bufs=2, space="PSUM") as psA, \
             tc.tile_pool(name="psT", bufs=2, space="PSUM") as psT, \
             tc.tile_pool(name="psS", bufs=1, space="PSUM") as psS, \
             tc.tile_pool(name="psB", bufs=3, space="PSUM") as psB:

            # ---------------- persistent tiles ----------------
            XS0 = per.tile([128, RX * W], FP16, tag="XS0")
            XS1 = per.tile([128, RX * W], FP16, tag="XS1")
            w1a = per.tile([128, CC], FP16, tag="w1a")
            w1b = per.tile([128, CC], FP16, tag="w1b")
            b1t = per.tile([1, CC], FP16, tag="b1t")
            w2t = per.tile([CC, NTAP * 40], FP16, tag="w2t")
            b2t = per.tile([1, 40], FP16, tag="b2t")
            ones = per.tile([1, 6 * W], FP16, tag="ones")
            idt16 = per.tile([128, 128], FP16, tag="idt16")
            idt = per.tile([128, 128], F32, tag="idt")
            c64t = per.tile([64, 322], F32, tag="c64t")
            shfT = per.tile([128, 5 * 128], FP16, tag="shfT")
            idxT = per.tile([128, 80], I16, tag="idxT")
            rmt = per.tile([CC, RX], FP16, tag="rmt")
            XPB = per.tile([128, RX * 256], FP16, tag="XPB")

            compp = per.tile([CC, RX * (W + 2)], FP16, tag="compp")
            EO = per.tile([40, RM * W + 2], FP16, tag="EO")
            mTE1 = per.tile([64, RM * 25], FP16, tag="mTE1")
            sRT1 = per.tile([64, RM], F32, tag="sRT1")
            tmpOff = per.tile([64, RM * 8], F32, tag="tmpOff")
            MSN = [per.tile([128, RM * 15], FP16, name=f"MSN{d}")
                   for d in range(3)]
            OYC = per.tile([64, 64], F32, tag="OYC")
            OXC = per.tile([64, 64], F32, tag="OXC")
            TRIYf = per.tile([64, 192], F32, tag="TRIYf")
            TRIXf = per.tile([64, 192], F32, tag="TRIXf")
            TRIY = per.tile([128, 192], FP16, tag="TRIY")
            TRIX = per.tile([128, 192], FP16, tag="TRIX")
            tmpW = per.tile([64, 192], F32, tag="tmpW")
            INNER = per.tile([128, NP * GH * 20], FP16, tag="INNER")
            tmpI = per.tile([128, NP * GH * 20], FP16, tag="tmpI")
            tmpJ = per.tile([128, NP * GH * 20], FP16, tag="tmpJ")
            MWg = [per.tile([128, NP * GH * 20], FP16, name=f"MW{g}")
                   for g in range(NG)]
            DMg = [per.tile([128, NP * GH * 20], FP16, name=f"DM{g}")
                   for g in range(NG)]
            Bg = [[per.tile([128, GH * 256], FP16, name=f"B{g}_{P}")
                   for P in range(NP)] for g in range(NG)]
            OCg = [[per.tile([128, GH * 256], FP16, name=f"OC{g}_{ch}")
                    for ch in range(2)] for g in range(NG)]

            # views of the packed const tile
            yloT = c64t[:, 0:64]
            yhiT = c64t[:, 64:128]
            xloT = c64t[:, 128:129]
            xhiT = c64t[:, 129:130]
            dytT = c64t[:, 130:322]

            # ---------------- input DMAs ----------------
            xv3 = x_sl.rearrange("c r w -> c (r w)")
            nc.sync.dma_start(XS0[:], xv3[0:128, :])
            nc.sync.dma_start(XS1[:], xv3[128:256, :])
            nc.sync.dma_start(w1a[:], w1l[0:128, :])
            nc.sync.dma_start(w1b[:], w1l[128:256, :])
            nc.sync.dma_start(b1t[:], b1r[:])
            nc.scalar.dma_start(w2t[:], w2l[:])
            nc.scalar.dma_start(b2t[:], b2r[:])
            nc.scalar.dma_start(idt[:], ident[:])
            nc.sync.dma_start(rmt[:], rmask[:])
            nc.sync.dma_start(c64t[:], c64[:])
            nc.scalar.dma_start(XPB[:].rearrange("p (r c) -> p r c", r=RX),
                                xpair.rearrange("r p c -> p r c"))
            nc.scalar.dma_start(shfT[:], shf[:])
            nc.scalar.dma_start(idxT[:], idxc[:])
            nc.vector.memset(ones[:], 1.0)

            # ---------------- conv1 (1x1) ----------------
            nc.vector.memset(compp[:], 0.0)
            cpv = compp[:].rearrange("p (r w) -> p r w", r=RX)
            xs0v = XS0[:].rearrange("p (r w) -> p r w", r=RX)
            xs1v = XS1[:].rearrange("p (r w) -> p r w", r=RX)
            for i in range(4):
                r0 = i * 5
                p1 = psA.tile([CC, 5 * W], F32, tag="cv", name="p1")
                nc.tensor.matmul(p1[:], w1a[:],
                                 xs0v[:, r0:r0 + 5, :], start=True, stop=False)
                nc.tensor.matmul(p1[:], w1b[:],
                                 xs1v[:, r0:r0 + 5, :], start=False, stop=False)
                nc.tensor.matmul(p1[:], b1t[:], ones[:, 0:5 * W],
                                 start=False, stop=True)
                nc.scalar.activation(
                    cpv[:, r0:r0 + 5, 1:65],
                    p1[:].rearrange("p (r w) -> p r w", r=5), ACTF.Copy)
            # zero comp rows outside global [0, H) (per-core 0/1 row mask)
            nc.vector.tensor_tensor(
                cpv[:, :, 1:65],
                cpv[:, :, 1:65],
                rmt[:].unsqueeze(2).broadcast_to([CC, RX, W]),
                op=ALU.mult)

            # ---------------- conv2 (3x3) + exp + off ----------------
            w2v = w2t[:].rearrange("p (t o) -> p t o", t=NTAP)
            eov = EO[:]  # [40, RM*W+2]; data cols at offset 1
            eo25 = EO[0:25, 1:1 + RM * W].rearrange("p (r w) -> p r w", r=RM)
            eo8 = EO[32:40, 1:1 + RM * W].rearrange("p (r w) -> p r w", r=RM)
            for i in range(3):
                r0 = i * 6
                p2 = psA.tile([40, 6 * W], F32, tag="cv", name="p2")
                for t in range(NTAP):
                    dy, dx = t // 3, t % 3
                    nc.tensor.matmul(
                        p2[:].rearrange("p (r w) -> p r w", r=6),
                        w2v[:, t, :],
                        cpv[:, r0 + dy:r0 + dy + 6, dx:dx + W],
                        start=(t == 0), stop=False)
                nc.tensor.matmul(p2[:], b2t[:], ones[:],
                                 start=False, stop=True)
                nc.scalar.activation(
                    eo25[:, r0:r0 + 6, :],
                    p2[0:25, :].rearrange("p (r w) -> p r w", r=6),
                    ACTF.Exp)
                nc.scalar.activation(
                    eo8[:, r0:r0 + 6, :],
                    p2[32:40, :].rearrange("p (r w) -> p r w", r=6),
                    ACTF.Copy)

            # ------- pair transposes: offsets first, then mask rows -------
            nc.vector.tensor_copy(idt16[:], idt[:])
            for r2 in range(RM // 2):
                r = 2 * r2
                po = psT.tile([128, 8], FP16, tag="ptc", name="po")
                nc.tensor.transpose(po[:],
                                    eov[32:40, 1 + r * W:1 + (r + 2) * W],
                                    idt16[32:40, 32:40])
                nc.vector.tensor_copy(tmpOff[:, r * 8:(r + 1) * 8],
                                      po[0:64, :])
                nc.vector.tensor_copy(tmpOff[:, (r + 1) * 8:(r + 2) * 8],
                                      po[64:128, :])
            for r2 in range(RM // 2):
                r = 2 * r2
                pt = psT.tile([128, 25], FP16, tag="ptc", name="pt")
                nc.tensor.transpose(pt[:],
                                    eov[0:25, 1 + r * W:1 + (r + 2) * W],
                                    idt16[0:25, 0:25])
                nc.scalar.activation(mTE1[:, r * 25:(r + 1) * 25],
                                     pt[0:64, :], ACTF.Copy)
                nc.scalar.activation(mTE1[:, (r + 1) * 25:(r + 2) * 25],
                                     pt[64:128, :], ACTF.Copy)

            # ---------------- softmax normalizer 1/max(sum,1) ----------------
            nc.vector.tensor_reduce(
                sRT1[:].unsqueeze(2),
                mTE1[:].rearrange("p (r c) -> p r c", r=RM),
                axis=mybir.AxisListType.X, op=ALU.add)
            nc.vector.tensor_scalar_max(sRT1[:], sRT1[:], 1.0)
            nc.vector.reciprocal(sRT1[:], sRT1[:])

            # ------- MSN (center): j-packed normalized masks, (r, P, kj) ----
            mtv = mTE1[:].rearrange("p (r k) -> p r k", r=RM)
            srb = sRT1[:].unsqueeze(2).broadcast_to([64, RM, 5])
            msv1 = MSN[1][:].rearrange("p (r P k) -> p r P k", r=RM, P=NP)
            for P in range(NP):
                for j in range(2):
                    k0 = (2 * P + j) * 5 if P < 2 else 20
                    nc.vector.tensor_tensor(
                        msv1[64 * j:64 * j + 64, :, P, :],
                        mtv[:, :, k0:k0 + 5], srb, op=ALU.mult)
            # +-1 column shifts of the center mask via shift matmuls
            for (dxi, kj) in ((0, 3), (2, 1)):
                pm = psS.tile([128, RM * 15], F32, tag="pskj",
                              name=f"pmsn{dxi}")
                nc.tensor.matmul(pm[:], shfT[:, kj * 128:(kj + 1) * 128],
                                 MSN[1][:], start=True, stop=True)
                nc.scalar.activation(MSN[dxi][:], pm[:], ACTF.Copy)

            # ---------------- WGT: bilinear tri-weights ----------------
            # tmpOff[w, (r, q, xy, p)]; rows r=1..17 are h=0..16
            tov = tmpOff[:].rearrange("p (r q xy pp) -> p r q xy pp",
                                      r=RM, q=2, xy=2)
            oyv = tov[:, 1:1 + HB, :, 1, :].transpose([0, 1, 3, 2])
            oxv = tov[:, 1:1 + HB, :, 0, :].transpose([0, 1, 3, 2])
            oycv = OYC[:].rearrange("p (h pp q) -> p h pp q", h=HB, pp=2)
            oxcv = OXC[:].rearrange("p (h pp q) -> p h pp q", h=HB, pp=2)
            nc.vector.tensor_tensor(
                oycv, oyv,
                yloT.rearrange("p (h pp q) -> p h pp q", h=HB, pp=2),
                op=ALU.max)
            nc.vector.tensor_tensor(
                oycv, oycv,
                yhiT.rearrange("p (h pp q) -> p h pp q", h=HB, pp=2),
                op=ALU.min)
            nc.vector.tensor_tensor(
                oxcv, oxv,
                xloT.unsqueeze(2).unsqueeze(3).broadcast_to([64, HB, 2, 2]),
                op=ALU.max)
            nc.vector.tensor_tensor(
                oxcv, oxcv,
                xhiT.unsqueeze(2).unsqueeze(3).broadcast_to([64, HB, 2, 2]),
                op=ALU.min)
            for (trif, tri16, oc) in ((TRIYf, TRIY, OYC), (TRIXf, TRIX, OXC)):
                ocb = oc[:].unsqueeze(1).broadcast_to([64, 3, 64])
                nc.vector.tensor_tensor(
                    tmpW[:].rearrange("p (d f) -> p d f", d=3), ocb,
                    dytT.rearrange("p (d f) -> p d f", d=3),
                    op=ALU.subtract)
                nc.vector.tensor_scalar(trif[:], tmpW[:], -1.0, None,
                                        op0=ALU.mult)
                nc.vector.tensor_tensor(trif[:], trif[:], tmpW[:], op=ALU.max)
                nc.vector.tensor_scalar(trif[:], trif[:], -1.0, 1.0,
                                        op0=ALU.mult, op1=ALU.add)
                nc.vector.tensor_scalar(trif[:], trif[:], 0.0, None,
                                        op0=ALU.max)
                nc.vector.tensor_copy(tri16[0:64, :], trif[:])
                nc.vector.tensor_copy(tri16[64:128, :], trif[:])

            # ---------------- pipelined back half (per h-group) ----------
            trxv = TRIX[:].rearrange("p (d h e) -> p d h e", d=3, h=HB)
            tryv = TRIY[:].rearrange("p (d h e) -> p d h e", d=3, h=HB)
            inv = INNER[:].rearrange("p (h pk e) -> p h pk e",
                                     h=GH, pk=NP * 5)
            ov = out_sl.rearrange("c r w -> c (r w)")
            for g in range(NG):
                hg0 = g * GH
                # --- MW: weighted upsampled mask, all 3 P-tiles per op ---
                # layout (h, (P,k), e=(p,q)) so every operand is <=3 free dims
                mwv = MWg[g][:].rearrange("p (h pk e) -> p h pk e",
                                          h=GH, pk=NP * 5)
                for dyi in range(3):
                    for dxi in range(3):
                        msl = MSN[dxi][:].rearrange(
                            "p (r pk) -> p r pk", r=RM)[
                            :, hg0 + dyi:hg0 + dyi + GH] \
                            .unsqueeze(3).broadcast_to([128, GH, 15, 4])
                        txl = trxv[:, dxi, hg0:hg0 + GH] \
                            .unsqueeze(2).broadcast_to([128, GH, 15, 4])
                        dst = inv if dxi == 0 else tmpI[:].rearrange(
                            "p (h pk e) -> p h pk e", h=GH, pk=15)
                        nc.vector.tensor_tensor(dst, txl, msl, op=ALU.mult)
                        if dxi > 0:
                            nc.vector.tensor_tensor(INNER[:], INNER[:],
                                                    tmpI[:], op=ALU.add)
                    tyl = tryv[:, dyi, hg0:hg0 + GH] \
                        .unsqueeze(2).broadcast_to([128, GH, 15, 4])
                    dst = mwv if dyi == 0 else tmpJ[:].rearrange(
                        "p (h pk e) -> p h pk e", h=GH, pk=15)
                    nc.vector.tensor_tensor(dst, tyl, inv, op=ALU.mult)
                    if dyi > 0:
                        nc.vector.tensor_tensor(MWg[g][:], MWg[g][:],
                                                tmpJ[:], op=ALU.add)

                # --- DM: kj-shift via constant shift-matrix matmuls ---
                mwk = MWg[g][:].rearrange("p (h P k e) -> p h P k e",
                                          h=GH, P=NP, k=5)
                dmk = DMg[g][:].rearrange("p (P h k e) -> p P h k e",
                                          P=NP, h=GH, k=5)
                for kj in range(5):
                    ps = psS.tile([128, NP * GH * 4], F32, tag="pskj",
                                  name=f"ps{g}_{kj}")
                    psv = ps[:].rearrange("p (h P e) -> p h P e",
                                          h=GH, P=NP)
                    nc.tensor.matmul(psv, shfT[:, kj * 128:(kj + 1) * 128],
                                     mwk[:, :, :, kj, :],
                                     start=True, stop=True)
                    nc.scalar.activation(dmk[:, :, :, kj, :],
                                         psv.transpose([0, 2, 1, 3]),
                                         ACTF.Copy)

                # --- B scatter (gpsimd) ---
                for P in range(NP):
                    ch = 128 if P < 2 else 64
                    nc.gpsimd.local_scatter(
                        Bg[g][P][0:ch, :],
                        DMg[g][0:ch, P * GH * 20:(P + 1) * GH * 20],
                        idxT[0:ch, :], ch, GH * 256, GH * 20)

                # --- CARAFE banded matmuls + evac ---
                for hh in range(GH):
                    h = hg0 + hh
                    for ch in range(2):
                        pb = psB.tile([128, 256], F32, tag="pb",
                                      name=f"pb{h}_{ch}")
                        cs = ch * 128
                        nc.tensor.matmul(
                            pb[:], XPB[:, h * 256 + cs:h * 256 + cs + 128],
                            Bg[g][0][:, hh * 256:(hh + 1) * 256],
                            start=True, stop=False)
                        nc.tensor.matmul(
                            pb[:],
                            XPB[:, (h + 2) * 256 + cs:(h + 2) * 256 + cs + 128],
                            Bg[g][1][:, hh * 256:(hh + 1) * 256],
                            start=False, stop=False)
                        nc.tensor.matmul(
                            pb[:],
                            XPB[0:64, (h + 4) * 256 + cs:(h + 4) * 256 + cs + 128],
                            Bg[g][2][0:64, hh * 256:(hh + 1) * 256],
                            start=False, stop=True)
                        nc.scalar.activation(
                            OCg[g][ch][:, hh * 256:(hh + 1) * 256],
                            pb[:], ACTF.Copy)

                # --- store this group's output rows ---
                csl = slice(g * GH * 256, (g + 1) * GH * 256)
                nc.sync.dma_start(ov[0:128, csl], OCg[g][0][:])
                nc.sync.dma_start(ov[128:256, csl], OCg[g][1][:])

            if _cache.get("debug"):
                nc.sync.dma_start(dbg["d_mTE0"], mTE1[:])
                nc.sync.dma_start(dbg["d_sRT0"], sRT1[:])
                nc.sync.dma_start(dbg["d_tmpOff"], tmpOff[:])
                nc.sync.dma_start(dbg["d_TRIY"], TRIY[:])
                nc.sync.dma_start(dbg["d_TRIX"], TRIX[:])
                nc.sync.dma_start(dbg["d_MSN1"], MSN[1][:])
                nc.sync.dma_start(dbg["d_MSN0"], MSN[0][:])

    nc.compile()
    return nc


def _consts(n, qh):
    h0 = qh * HB
    hg = h0 + np.arange(HB, dtype=np.float32)          # global h per local h
    ylo = np.broadcast_to(np.repeat(-hg, 4)[None, :], (64, 64))
    yhi = np.broadcast_to(np.repeat(63.0 - hg, 4)[None, :], (64, 64))
    wv = np.arange(W, dtype=np.float32)
    xlo = (-wv)[:, None]
    xhi = (63.0 - wv)[:, None]
    dyv = np.array([-1.0, 0.0, 1.0], np.float32)
    dyt = np.broadcast_to(np.repeat(dyv, 64)[None, :], (64, 192))
    c64 = np.concatenate([ylo, yhi, xlo, xhi, dyt], axis=1).astype(np.float32)
    # shift matrices: SHF[kj][p, m] = 1 iff same 64-block, m%64 = p%64 + kj-2
    shf = np.zeros((128, 5, 128), np.float16)
    for kj in range(5):
        for p in range(128):
            m = p + kj - 2
            if p // 64 == m // 64 and 0 <= m < 128:
                shf[p, kj, m] = 1.0
    shf = shf.reshape(128, 5 * 128)
    # scatter indices: partition (j,w'), slot (hh, kj, p, q) ->
    # col hh*256 + p*128 + q*64 + (w'+2-kj), or -1 if w out of range
    idx = np.full((128, 80), -1, np.int16)
    for pp in range(128):
        wp = pp % 64
        for hh in range(4):
            for kj in range(5):
                for p in range(2):
                    for q in range(2):
                        w = wp + 2 - kj
                        if 0 <= w < W:
                            idx[pp, hh * 20 + kj * 4 + p * 2 + q] = (
                                hh * 256 + p * 128 + q * 64 + w)
    rm = np.zeros((CC, RX), np.float16)
    for r in range(RX):
        g = h0 - 2 + r
        rm[:, r] = 1.0 if 0 <= g < H else 0.0
    return dict(c64=np.ascontiguousarray(c64), shf=shf, idxc=idx, rmask=rm)


def _prep_inmaps(x, w_comp, b_comp, w_off, b_off, w_ker, b_ker):
    x = np.asarray(x, np.float32)
    w_comp = np.asarray(w_comp, np.float32)
    b_comp = np.asarray(b_comp, np.float32)
    w_off = np.asarray(w_off, np.float32)
    b_off = np.asarray(b_off, np.float32)
    w_ker = np.asarray(w_ker, np.float32)
    b_ker = np.asarray(b_ker, np.float32)

    w1l = w_comp.reshape(CC, C).T.astype(np.float16)
    perm = [xy * 4 + p * 2 + q for q in range(2) for xy in range(2)
            for p in range(2)]
    w2 = np.zeros((40, CC, 3, 3), np.float32)
    b2 = np.zeros((40,), np.float32)
    w2[0:25] = w_ker
    b2[0:25] = b_ker
    w2[32:40] = w_off[perm]
    b2[32:40] = b_off[perm]
    w2l = np.ascontiguousarray(
        w2.transpose(1, 2, 3, 0).reshape(CC, NTAP * 40)).astype(
        np.float16)   # [cc, (tap, oc)]
    ident = np.eye(128, dtype=np.float32)

    in_maps = []
    for core in range(8):
        n, qh = core // QH, core % QH
        h0 = qh * HB
        xs = np.zeros((C, RX, W), np.float16)
        lo, hi = h0 - 2, h0 + HB + 2
        slo, shi = max(lo, 0), min(hi, H)
        xs[:, slo - lo:shi - lo] = x[n, :, slo:shi].astype(np.float16)
        xt = np.ascontiguousarray(xs.transpose(2, 1, 0))
        xp = np.zeros((RX, 128, 256), np.float16)
        xp[:, 0:64, :] = xt.transpose(1, 0, 2)
        xp[:RX - 1, 64:128, :] = xt.transpose(1, 0, 2)[1:]
        im = dict(x_sl=xs, xpair=xp, w1l=w1l,
                  b1r=b_comp[None, :].astype(np.float16),
                  b2r=b2[None, :].astype(np.float16),
                  w2l=w2l, ident=ident,
                  **_consts(n, qh))
        in_maps.append(im)
    return in_maps


def _post(o):
    """out_sl cols are (q, w)-ordered fp16; reorder to (w, q) f32."""
    o = o.astype(np.float32).reshape(C, 2 * HB, 2, W)
    return o.transpose(0, 1, 3, 2).reshape(C, 2 * HB, WOUT)


def kernel(x, w_comp, b_comp, w_off, b_off, w_ker, b_ker):
    if "nc" not in _cache:
        _cache["nc"] = _build()
    nc = _cache["nc"]
    in_maps = _prep_inmaps(x, w_comp, b_comp, w_off, b_off, w_ker, b_ker)

    res = run_bass_kernel_spmd(nc, in_maps, core_ids=list(range(8)),
                               trace=bool(os.environ.get("DLU_TRACE")))
    _cache["last_res"] = res
    out = np.zeros((N, C, HOUT, WOUT), np.float32)
    for core in range(8):
        n, qh = core // QH, core % QH
        out[n, :, 2 * qh * HB:2 * (qh + 1) * HB] = _post(
            res.results[core]["out_sl"])
    return out
